# revision 12
# baseline (speedup 1.0000x reference)
"""Trainium2 Bass kernel for nn_BranchGCN (gnn_message_passing), v2.

Stage A -- node-model-parallel (4 nodes/core). Per node: stream W_branch fp16
  on rotating DMA queues, 64 small matmuls (d-major PSUM), Act-fused LeakyReLU
  copy to fp16, one 3-matmul combine (Wl-fold / root / bias) and a single
  fp32 store of [x, y, u] rows. xx2 and fp16 splits are host relayout.
Stage B -- row-sharded KNN EdgeConv via slot-centroid coarse selection:
  128 slots of 16 consecutive points per sample; per 128-query tile one K=4
  matmul scores all slots, top-8 slots by max8/max_index, one batched 2-DMA
  index relayout for all 32 tiles, per-tile 1024-row gather of candidate
  [x, xx2, y] rows, exact fp32 pd recompute (3 fused ops), threshold at the
  8th largest, penalised max-y via tensor_tensor_reduce, epilogue + leaky.
"""

import sys
import numpy as np

sys.path.insert(0, "/opt/trn_rl_repo")

from contextlib import ExitStack

import concourse.tile as tile
from concourse import bacc, bass, mybir
from concourse.bass_utils import run_bass_kernel_spmd

FP = mybir.dt.float32
FP16 = mybir.dt.float16
U16 = mybir.dt.uint16
I16 = mybir.dt.int16
ALU = mybir.AluOpType
AF = mybir.ActivationFunctionType

B, NODE, DEG, K = 16, 32, 64, 8
IN_F, OUT_F, SUP = 128, 3, 10
FEATS = [96, 256, 256, 256, 128, 128]
SIZES = [1, 2, 4, 8, 16, 32]
NCORES = 8
NLOC = NODE // NCORES          # 4 nodes per core
N = NODE * DEG                 # 2048 graph rows
RLOC = NLOC * DEG              # 256 rows per core
NP_CORE = B * RLOC             # 4096 points per core
NCH = [(f + 127) // 128 for f in FEATS]
BIG = 60000.0

G = 64                         # slot size (consecutive points)
NS = N // G                    # 32 slots per sample
TS = 2                         # slots expanded per query
NT = 2 * B                     # 32 query tiles per core
CAND = TS * G                  # 128 candidates per query

# column offsets into blob32: tlT blocks then wr blocks
_TL_W = [NCH[i] * NLOC * B for i in range(6)]
_WR_W = [NCH[i] * OUT_F for i in range(6)]
_TL_OFF = np.cumsum([0] + _TL_W).tolist()
_WR_OFF = (np.cumsum([0] + _WR_W) + _TL_OFF[6]).tolist()
BLOB32_W = _WR_OFF[6]
# blob16 columns: t5h (nl,b)=64 | wbig 16 | abig 16 | bsel 16 | biasu 64
B16_T5, B16_WBIG, B16_ABIG, B16_BSEL, B16_BIASU = 0, 64, 80, 96, 112
BLOB16_W = 176


# --------------------------------------------------------------------------
# Stage A
# --------------------------------------------------------------------------
def build_stage_a():
    nc = bacc.Bacc(None)
    blob32 = nc.declare_dram_parameter("blob32", [128, BLOB32_W], FP,
                                       isOutput=False)
    blob16 = nc.declare_dram_parameter("blob16", [128, BLOB16_W], FP16,
                                       isOutput=False)
    wb = nc.declare_dram_parameter("wb", [NLOC, IN_F, DEG * IN_F], FP16,
                                   isOutput=False)
    # rows 0-2 x, 4-6 y, 8-10 u; cols per node (g, dl, b) d-major
    xout = nc.declare_dram_parameter("xout", [16, NLOC * B * DEG], FP,
                                     isOutput=True)

    with tile.TileContext(nc) as tc, ExitStack() as ctx:
        sbp = ctx.enter_context(tc.tile_pool(name="sbuf", bufs=1))
        wbpool = ctx.enter_context(tc.tile_pool(name="wbuf", bufs=3))
        psp = ctx.enter_context(tc.tile_pool(name="psum", bufs=2,
                                             space="PSUM"))
        pbp = ctx.enter_context(tc.tile_pool(name="psumb", bufs=2,
                                             space="PSUM"))
        pxp = ctx.enter_context(tc.tile_pool(name="psumx", bufs=2,
                                             space="PSUM"))

        b32 = sbp.tile([128, BLOB32_W], FP)
        nc.sync.dma_start(out=b32[:], in_=blob32[:])
        b16 = sbp.tile([128, BLOB16_W], FP16)
        nc.scalar.dma_start(out=b16[:], in_=blob16[:])

        # ---- root aggregation -> rootT fp16 (3, nl, b)
        proot = psp.tile([OUT_F, NLOC * B], FP, tag="proot")
        steps = [(i, c) for i in range(6) for c in range(NCH[i])]
        for si, (i, c) in enumerate(steps):
            nc.tensor.matmul(
                out=proot[:],
                lhsT=b32[:, _WR_OFF[i] + c * OUT_F:_WR_OFF[i] + (c + 1) * OUT_F],
                rhs=b32[:, _TL_OFF[i] + c * NLOC * B:
                        _TL_OFF[i] + (c + 1) * NLOC * B],
                start=(si == 0), stop=(si == len(steps) - 1))
        rootT = sbp.tile([OUT_F, NLOC, B], FP16)
        nc.scalar.activation(out=rootT[:],
                             in_=proot[:].rearrange("p (nl b) -> p nl b",
                                                    nl=NLOC),
                             func=AF.Copy)

        t5v = b16[:, B16_T5:B16_T5 + NLOC * B].rearrange(
            "p (nl b) -> p nl b", nl=NLOC)
        wbig_sb = b16[:, B16_WBIG:B16_WBIG + 16]
        abig_sb = b16[0:OUT_F, B16_ABIG:B16_ABIG + 16]
        bsel_sb = b16[0:OUT_F, B16_BSEL:B16_BSEL + 16]
        biasu_sb = b16[0:OUT_F, B16_BIASU:B16_BIASU + DEG]

        CW = B * DEG
        qs = [nc.sync, nc.scalar, nc.gpsimd]
        for nl in range(NLOC):
            wbt = wbpool.tile([IN_F, DEG * IN_F], FP16, tag="wbt")
            h = DEG * IN_F // 2
            qa, qb = [(0, 2), (1, 0), (2, 1), (0, 2)][nl]
            qs[qa].dma_start(out=wbt[:, 0:h], in_=wb[nl][:, 0:h])
            qs[qb].dma_start(out=wbt[:, h:2 * h],
                             in_=wb[nl][:, h:2 * h])
            branchT = wbpool.tile([IN_F, 2, 32, B], FP16,
                                   tag="branchT")
            for g in range(2):
                pb = pbp.tile([IN_F, 32, B], FP, tag="pbranch")
                for dl in range(32):
                    d = g * 32 + dl
                    nc.tensor.matmul(out=pb[:, dl, :],
                                     lhsT=wbt[:, d * 128:(d + 1) * 128],
                                     rhs=t5v[:, nl, :],
                                     start=True, stop=True)
                if g == 0:
                    nc.vector.tensor_copy(out=branchT[:, g], in_=pb[:])
                else:
                    nc.scalar.activation(out=branchT[:, g], in_=pb[:],
                                         func=AF.Copy)
                nc.vector.scalar_tensor_tensor(
                    out=branchT[:, g], in0=branchT[:, g], scalar=0.2,
                    in1=branchT[:, g], op0=ALU.mult, op1=ALU.max)
            pxo = pxp.tile([16, 2, 32, B], FP, tag="pxo")
            for g in range(2):
                nc.tensor.matmul(
                    out=pxo[:, g], lhsT=wbig_sb,
                    rhs=branchT[:, g].rearrange("p dl b -> p (dl b)"),
                    start=True, stop=False)
                nc.tensor.matmul(
                    out=pxo[:, g], lhsT=abig_sb,
                    rhs=rootT[:, nl, :].unsqueeze(1).to_broadcast(
                        [OUT_F, 32, B]),
                    start=False, stop=False)
                nc.tensor.matmul(
                    out=pxo[:, g], lhsT=bsel_sb,
                    rhs=biasu_sb[:, g * 32:(g + 1) * 32].unsqueeze(2)
                        .to_broadcast([OUT_F, 32, B]),
                    start=False, stop=True)
            xo = wbpool.tile([16, DEG * B], FP, tag="xo")
            nc.scalar.activation(
                out=xo[:], in_=pxo[:].rearrange("p g d b -> p (g d b)"),
                func=AF.Copy)
            qs[(2 * nl) % 3].dma_start(out=xout[:, nl * CW:(nl + 1) * CW],
                                       in_=xo[:])
    return nc


# --------------------------------------------------------------------------
# Stage B
# --------------------------------------------------------------------------
NBLK = (NT + 2) // 3  # 11 lhsT column blocks


def build_stage_b():
    nc = bacc.Bacc(None)
    from concourse import library_config

    # lhsT packs: tile t at partitions 32*(t%3)+0..3, block t//3
    qT = nc.declare_dram_parameter("qT", [96, NBLK * 128], FP16,
                                   isOutput=False)
    # slot tables: rows 32j+0..3 = [c0,c1,c2,-h] of sample b (all j)
    sT = nc.declare_dram_parameter("sT", [96, B * NS], FP16, isOutput=False)
    # per-(partition, tile): xn0..2, pad, u0..2, pad
    qd = nc.declare_dram_parameter("qd", [128, NT * 8], FP, isOutput=False)
    ptabs = [nc.declare_dram_parameter(f"ptab{b}", [NS, G * 8], FP,
                                       isOutput=False) for b in range(B)]
    outc = nc.declare_dram_parameter("outc", [128, NT * OUT_F], FP,
                                     isOutput=True)

    with tile.TileContext(nc) as tc, ExitStack() as ctx:
        sbp = ctx.enter_context(tc.tile_pool(name="sbuf", bufs=1))
        lop = ctx.enter_context(tc.tile_pool(name="loop", bufs=12))
        gp = ctx.enter_context(tc.tile_pool(name="gpool", bufs=8))
        dramp = ctx.enter_context(tc.tile_pool(name="dram", bufs=1,
                                               space="DRAM"))
        pspd = ctx.enter_context(tc.tile_pool(name="pspd", bufs=8,
                                              space="PSUM"))

        nc.gpsimd.load_library(library_config.mlp)
        qT_sb = sbp.tile([96, NBLK, 128], FP16)
        nc.sync.dma_start(out=qT_sb[:],
                          in_=qT[:].rearrange("p (k c) -> p k c", k=NBLK))
        sT_sb = sbp.tile([96, B, NS], FP16)
        nc.scalar.dma_start(out=sT_sb[:],
                            in_=sT[:].rearrange("p (b s) -> p b s", b=B))
        qd_sb = sbp.tile([128, NT, 8], FP)
        nc.sync.dma_start(out=qd_sb[:],
                            in_=qd[:].rearrange("p (t v) -> p t v", t=NT))

        idx2q = []
        for _h in range(4):
            i2q = sbp.tile([128, NT // 4, K], U16, tag=f"i2q{_h}")
            idx2q.append(i2q)
        res = sbp.tile([128, NT, OUT_F], FP)

        idxs = sbp.tile([128, NT, TS, 8], I16)
        ysPair = {}
        HT = NT // 4

        def phase1(t):
            b, j, blk = t // 2, t % 3, t // 3
            ps = pspd.tile([128, NS], FP, tag="ps")
            nc.tensor.matmul(out=ps[:],
                             lhsT=qT_sb[32 * j:32 * j + 4, blk, :],
                             rhs=sT_sb[32 * j:32 * j + 4, b, :],
                             start=True, stop=True)
            cs = lop.tile([128, NS], FP16, tag="cs")
            nc.scalar.activation(out=cs[:], in_=ps[:], func=AF.Copy)
            top8 = lop.tile([128, K], FP16, tag="top8")
            nc.vector.max(out=top8[:], in_=cs[:])
            nc.vector.max_index(out=idx2q[t // HT][:, t % HT, :],
                                in_max=top8[:], in_values=cs[:])

        def bounce(h):
            # scrD[q, r, c=(t,j)] = idx2[16q+r, t0+t, j] for half h
            t0 = h * HT
            scrD = dramp.tile([8, 16, HT, TS], U16, tag=f"scrD{h}")
            nc.scalar.dma_start(
                out=scrD[:].rearrange("q r t j -> (q r) t j"),
                in_=idx2q[h][:, :, 0:TS])
            idxsA = sbp.tile([16, 8, HT * TS], U16, tag=f"iA{h}")
            nc.sync.dma_start(out=idxsA[:],
                              in_=scrD[:].rearrange("q r t j -> r q (t j)"))
            idxsB = sbp.tile([16, HT, TS, 8], U16, tag=f"iB{h}")
            nc.vector.tensor_copy(
                out=idxsB[:].rearrange("r t j q -> r q t j"),
                in_=idxsA[:].rearrange("r q (t j) -> r q t j", t=HT))
            scr2 = dramp.tile([16, HT * TS * 8], U16, tag=f"scr2{h}")
            nc.scalar.dma_start(
                out=scr2[:], in_=idxsB[:].rearrange("r t j q -> r (t j q)"))
            nc.sync.dma_start(
                out=idxs[:, t0:t0 + HT].rearrange("p t j q -> p (t j q)"),
                in_=scr2[:].rearrange("r c -> (r c)").unsqueeze(0)
                    .to_broadcast([8, 16 * HT * TS * 8]).bitcast(I16))

        def phase2(t):
            b = t // 2
            g = gp.tile([128, TS, G * 8], FP, tag="g")
            nc.gpsimd.dma_gather(g[:], ptabs[b][:],
                                 idxs[:, t].rearrange("p j q -> p (j q)"),
                                 128 * TS, 128 * TS, G * 8)
            gv = g[:].rearrange("p k (e v) -> p k e v", v=8)
            s1 = lop.tile([128, TS, G], FP, tag="s1")
            nc.vector.scalar_tensor_tensor(
                out=s1[:], in0=gv[:, :, :, 0], scalar=qd_sb[:, t, 0:1],
                in1=gv[:, :, :, 3], op0=ALU.mult, op1=ALU.subtract)
            s2 = lop.tile([128, TS, G], FP, tag="s2")
            nc.vector.scalar_tensor_tensor(
                out=s2[:], in0=gv[:, :, :, 1], scalar=qd_sb[:, t, 1:2],
                in1=s1[:], op0=ALU.mult, op1=ALU.add)
            pdc = lop.tile([128, TS, G], FP, tag="pdc")
            nc.vector.scalar_tensor_tensor(
                out=pdc[:], in0=gv[:, :, :, 2], scalar=qd_sb[:, t, 2:3],
                in1=s2[:], op0=ALU.mult, op1=ALU.add)
            t8 = lop.tile([128, K], FP, tag="t8")
            nc.vector.max(out=t8[:],
                          in_=pdc[:].rearrange("p k e -> p (k e)"))
            r = lop.tile([128, TS * G], FP, tag="r")
            nc.scalar.activation(out=r[:],
                                 in_=pdc[:].rearrange("p k e -> p (k e)"),
                                 func=AF.Relu, bias=t8[:, 7:8], scale=-1.0)
            rv = r[:].rearrange("p (k e) -> p k e", k=TS).unsqueeze(3) \
                .to_broadcast([128, TS, G, OUT_F])
            if t % 2 == 0:
                ysP = lop.tile([128, 2, OUT_F, TS, G], FP16, tag="ysP")
                ysPair[t // 2] = ysP
            ysP = ysPair[t // 2]
            nc.gpsimd.tensor_tensor(
                out=ysP[:, t % 2].rearrange("p c k e -> p k e c"),
                in0=gv[:, :, :, 4:7], in1=rv, op=ALU.subtract)
            if t % 2 == 1:
                nc.vector.tensor_reduce(
                    out=res[:, t - 1:t + 1, :].rearrange("p t c -> p (t c)"),
                    in_=ysP[:].rearrange("p t c k e -> p (t c) (k e)"),
                    axis=mybir.AxisListType.X, op=ALU.max)

        for t in range(HT):
            phase1(t)
        bounce(0)
        for t in range(HT, 2 * HT):
            phase1(t)
        bounce(1)
        for t in range(2 * HT, 3 * HT):
            phase1(t)
        for t in range(HT):
            phase2(t)
        bounce(2)
        for t in range(3 * HT, NT):
            phase1(t)
        for t in range(HT, 2 * HT):
            phase2(t)
        bounce(3)
        for t in range(2 * HT, NT):
            phase2(t)

        # ---- epilogue: += u, leaky, store
        nc.vector.scalar_tensor_tensor(
            out=res[:], in0=res[:], scalar=0.0,
            in1=qd_sb[:, :, 4:7], op0=ALU.bypass, op1=ALU.add)
        nc.vector.scalar_tensor_tensor(
            out=res[:], in0=res[:], scalar=0.2,
            in1=res[:], op0=ALU.mult, op1=ALU.max)
        nc.sync.dma_start(out=outc[:],
                          in_=res[:].rearrange("p t o -> p (t o)"))
    return nc


# --------------------------------------------------------------------------
# Host orchestration
# --------------------------------------------------------------------------
_CACHE = {}
LAST_RESULTS = {}


def _programs():
    if "a" not in _CACHE:
        nca = build_stage_a()
        nca.compile()
        ncb = build_stage_b()
        ncb.compile()
        _CACHE["a"] = nca
        _CACHE["b"] = ncb
    return _CACHE["a"], _CACHE["b"]


def _weight_folds(inputs):
    c1w = np.asarray(inputs["c1w"], np.float32)
    c1b = np.asarray(inputs["c1b"], np.float32)
    c2w = np.asarray(inputs["c2w"], np.float32)
    c2b = np.asarray(inputs["c2b"], np.float32)
    M1 = c1w[:, :3].T @ c2w.T                      # (3, 3)
    M2 = c1w[:, 3:].T @ c2w.T                      # (3, 3)
    zc = (c1b @ c2w.T + c2b).reshape(3)
    Wl = (np.asarray(inputs["Wl1"], np.float32)
          @ np.asarray(inputs["Wl2"], np.float32))  # (128, 3)
    wbig = np.zeros((IN_F, 16), np.float32)
    wbig[:, 0:3] = Wl
    wbig[:, 4:7] = Wl @ M1
    wbig[:, 8:11] = Wl @ (M2 - M1)
    abig = np.zeros((OUT_F, 16), np.float32)
    abig[:, 0:3] = np.eye(3, dtype=np.float32)
    abig[:, 4:7] = M1
    abig[:, 8:11] = M2 - M1
    bsel = np.zeros((OUT_F, 16), np.float32)
    bsel[:, 8:11] = np.eye(3, dtype=np.float32)
    biasd = np.asarray(inputs["bias"], np.float32).reshape(DEG, OUT_F)
    biasu = np.ascontiguousarray((biasd + zc.reshape(1, 3)).T)  # (3, 64)
    return wbig, abig, bsel, biasu


def _stage_a_inmaps(inputs):
    trees = [np.asarray(inputs[f"t{i}"], np.float32) for i in range(6)]
    wrs = [np.asarray(inputs[f"Wr{i}"], np.float32) for i in range(6)]
    wbf = np.asarray(inputs["W_branch"], np.float32).astype(np.float16)
    wbig, abig, bsel, biasu = _weight_folds(inputs)
    t5 = trees[5]
    in_maps = []
    for c in range(NCORES):
        nodes = [NLOC * c + j for j in range(NLOC)]
        b32 = np.zeros((128, BLOB32_W), np.float32)
        for i in range(6):
            f, nch = FEATS[i], NCH[i]
            rows = [n * SIZES[i] // NODE for n in nodes]
            sl = trees[i][:, rows, :].transpose(2, 1, 0).reshape(f, NLOC * B)
            slp = np.zeros((nch * 128, NLOC * B), np.float32)
            slp[:f] = sl
            b32[:, _TL_OFF[i]:_TL_OFF[i + 1]] = (
                slp.reshape(nch, 128, NLOC * B).transpose(1, 0, 2)
                .reshape(128, nch * NLOC * B))
            wp = np.zeros((nch * 128, OUT_F), np.float32)
            wp[:f] = wrs[i]
            b32[:, _WR_OFF[i]:_WR_OFF[i + 1]] = (
                wp.reshape(nch, 128, OUT_F).transpose(1, 0, 2)
                .reshape(128, nch * OUT_F))
        b16 = np.zeros((128, BLOB16_W), np.float16)
        b16[:, B16_T5:B16_T5 + NLOC * B] = (
            t5[:, nodes, :].transpose(2, 1, 0).reshape(IN_F, NLOC * B)
            .astype(np.float16))
        b16[:, B16_WBIG:B16_WBIG + 16] = wbig.astype(np.float16)
        b16[0:3, B16_ABIG:B16_ABIG + 16] = abig.astype(np.float16)
        b16[0:3, B16_BSEL:B16_BSEL + 16] = bsel.astype(np.float16)
        b16[0:3, B16_BIASU:B16_BIASU + DEG] = biasu.astype(np.float16)
        m = {"blob32": b32, "blob16": b16,
             "wb": np.ascontiguousarray(wbf[nodes])}
        in_maps.append(m)
    return in_maps


def _stage_b_inmaps(inputs, xouts):
    # xouts: per-core (16, 4096) fp32, cols (nl, d, b) d-major
    xs = np.stack([np.asarray(x).reshape(16, NLOC, DEG, B) for x in xouts])
    # global arrays (16rows, B, N): n = (core nl d) -> core*256 + nl*64 + d
    allp = xs.transpose(1, 4, 0, 2, 3).reshape(16, B, N)
    x = allp[0:3]                      # (3, B, N)
    y = allp[4:7]
    u = allp[8:11]
    xx2 = 0.5 * np.sum(x * x, axis=0)  # (B, N)

    # slot tables: centroid + mean xx2
    xg = x.reshape(3, B, NS, G)
    cmean = xg.mean(axis=3)                            # (3, B, NS)
    hmean = xx2.reshape(B, NS, G).mean(axis=2)         # (B, NS)
    sTf = np.zeros((96, B, NS), np.float16)
    for j in range(3):
        sTf[32 * j + 0:32 * j + 3] = cmean.astype(np.float16)
        sTf[32 * j + 3] = (-hmean).astype(np.float16)
    sT = np.ascontiguousarray(sTf.reshape(96, B * NS))

    # ptabs: per sample [NS, G*8] = 16 pts x [x0,x1,x2,xx2,y0,y1,y2,0]
    ptabs = {}
    for b in range(B):
        pt = np.zeros((N, 8), np.float32)
        pt[:, 0:3] = x[:, b].T
        pt[:, 3] = xx2[b] * BIG
        pt[:, 4:7] = y[:, b].T
        ptabs[f"ptab{b}"] = np.ascontiguousarray(
            pt.reshape(NS, G * 8))

    in_maps = []
    for c in range(NCORES):
        # queries of tile t=(b,m): global rows c*256 + m*128 + p
        rows = c * RLOC + np.arange(RLOC)              # (256,)
        qx = x[:, :, rows]                             # (3, B, 256)
        qu = u[:, :, rows]
        # qT: tile t -> partitions 32*(t%3)+0..3, block t//3
        qTf = np.zeros((96, NBLK, 128), np.float16)
        qdf = np.zeros((128, NT, 8), np.float32)
        for t in range(NT):
            b, m, j, blk = t // 2, t % 2, t % 3, t // 3
            qrows = qx[:, b, m * 128:(m + 1) * 128]    # (3, 128)
            qTf[32 * j + 0:32 * j + 3, blk] = qrows.astype(np.float16)
            qTf[32 * j + 3, blk] = 1.0
            qdf[:, t, 0:3] = qrows.T * BIG
            qdf[:, t, 4:7] = qu[:, b, m * 128:(m + 1) * 128].T
        m_ = {"qT": np.ascontiguousarray(qTf.reshape(96, NBLK * 128)),
              "sT": sT,
              "qd": np.ascontiguousarray(qdf.reshape(128, NT * 8))}
        m_.update(ptabs)
        in_maps.append(m_)
    return in_maps


def kernel(**inputs):
    nca, ncb = _programs()
    core_ids = list(range(NCORES))

    ra = run_bass_kernel_spmd(nca, _stage_a_inmaps(inputs), core_ids)
    LAST_RESULTS["a"] = ra
    xouts = [np.asarray(ra.results[c]["xout"]) for c in range(NCORES)]

    rb = run_bass_kernel_spmd(ncb, _stage_b_inmaps(inputs, xouts),
                              core_ids)
    LAST_RESULTS["b"] = rb
    # outc [128, NT, 3]: partition p, tile t=(b, m), channel
    out = np.empty((B, N, OUT_F), np.float32)
    for c in range(NCORES):
        oc = np.asarray(rb.results[c]["outc"]).reshape(128, NT, OUT_F)
        oc = oc.transpose(1, 0, 2).reshape(B, 2, 128, OUT_F) \
            .reshape(B, RLOC, OUT_F)
        out[:, c * RLOC:(c + 1) * RLOC, :] = oc
    return out
